# revision 4
# baseline (speedup 1.0000x reference)
"""Trainium2 Bass kernel for nn_EntitiesIndexingHeadRuleBased (nms_detection).

v3: fp16 TT-form map pipeline (Pool has no ALU ops on real HW).

v2: engine-balanced, bf16-heavy rewrite.
  * GIoU is scale-invariant -> box chain runs on NORMALIZED coords (f32
    through the interval cancellations, bf16 after).
  * L1 chain needs pixel scale: centers kept f32 until after |.|, bf16 after.
  * cls chain: softmax packs in bf16, PE matmuls bf16 (3x faster), sqrt on ACT.
  * Work split per (map,chunk): ACT 7 ops (relu/abs/affine/sqrt),
    POOL(gpsimd) 8 stt combines, DVE 5 dual-scalar TS + 6 stt + 3 TT + recip,
    PE 2 bf16 matmuls.
  * Outputs written bf16, host converts to f32.

Sharding: batch 64 = 8 images x 8 cores, SPMD.
"""
import sys
sys.path.insert(0, '/opt/trn_rl_repo')

import numpy as np
import bass_rust
import concourse.bass as bass
import concourse.tile as tile
import concourse.tile as tile_mod
from concourse import mybir
from concourse import bass_utils
from concourse.masks import make_identity
from concourse.tile import TileContext

F32 = mybir.dt.float32
BF16 = mybir.dt.bfloat16
FP16 = mybir.dt.float16
AF = mybir.ActivationFunctionType
OP = mybir.AluOpType

B = 64
NE = 500
NR = 500
NC1 = 151
NCL = 150
P = 125
NCH = 4
N_CORES = 8
N_IMG = B // N_CORES

# ---------------------------------------------------------------------------
# Walrus workaround: instructions may carry at most one sync-wait command.
# ---------------------------------------------------------------------------

_MAXW = 1


def _patched_drain_and_barrier(self, tick_clock, wait_clock):
    ScopedClock = tile_mod.ScopedClock
    carrier = self.nc.sync.nop(nofuse=True)
    wait_clock.add_sem_waits(carrier.ins,
                             ScopedClock({None: tick_clock.global_clock}))
    si = carrier.ins.sync_info
    waits = list(si.on_wait) if si is not None else []
    if len(waits) > _MAXW:
        carrier.ins.sync_info = bass_rust.SyncInfo(
            on_wait=waits[:_MAXW], on_update=[])
        for i in range(_MAXW, len(waits), _MAXW):
            nop = self.nc.sync.nop(nofuse=True)
            nop.ins.sync_info = bass_rust.SyncInfo(
                on_wait=waits[i:i + _MAXW], on_update=[])
    self.nc.sync.drain()
    self.nc.all_engine_barrier()
    assert self.sems is not None
    popped = self.nc._tile_sem_poison_stack.pop()
    assert popped is self._sem_poison
    self.nc.clear_and_free_semaphores(list(self.sems.allocated().values()))
    self.nc.all_engine_barrier()


TileContext._drain_and_barrier = _patched_drain_and_barrier


def _split_waits(nc, maxw=_MAXW):
    for fn in nc.m.functions:
        for blk in fn.blocks:
            newl = []
            changed = False
            for ins in blk.instructions:
                si = ins.sync_info
                waits = list(si.on_wait) if si is not None else []
                if len(waits) > maxw:
                    changed = True
                    carried, rest = waits[:-maxw], waits[-maxw:]
                    for i in range(0, len(carried), maxw):
                        nop = mybir.InstNoOp(
                            name=f"{ins.name}-sw{i}",
                            sync_info=mybir.SyncInfo(
                                on_wait=carried[i:i + maxw], on_update=[]),
                            bass_nofuse=True,
                            engine=ins.engine,
                        )
                        newl.append(nop)
                    ins.sync_info = mybir.SyncInfo(
                        on_wait=rest, on_update=list(si.on_update))
                newl.append(ins)
            if changed:
                blk.instructions = newl


# ---------------------------------------------------------------------------
# Kernel builder
# ---------------------------------------------------------------------------

def _bcast(ap, p):
    return bass.AP(tensor=ap.tensor, offset=ap.offset,
                   ap=[[0, p]] + list(ap.ap[1:]))


def _build(n_img, split_waits=True):
    nc = bass.Bass("TRN2", target_bir_lowering=False)

    pb = nc.dram_tensor("pred_boxes", [n_img, NE, 4], F32, kind="ExternalInput")
    pl = nc.dram_tensor("pred_logits", [n_img, NE, NC1], F32, kind="ExternalInput")
    rol = nc.dram_tensor("pred_rel_obj_logits", [n_img, NR, NC1], F32, kind="ExternalInput")
    rsl = nc.dram_tensor("pred_rel_sub_logits", [n_img, NR, NC1], F32, kind="ExternalInput")
    rob = nc.dram_tensor("pred_rel_obj_box", [n_img, NR, 4], F32, kind="ExternalInput")
    rsb = nc.dram_tensor("pred_rel_sub_box", [n_img, NR, 4], F32, kind="ExternalInput")
    rv = nc.dram_tensor("pred_rel_vec", [n_img, NR, 4], F32, kind="ExternalInput")
    tsz = nc.dram_tensor("target_sizes", [n_img, 2], F32, kind="ExternalInput")
    out_s = nc.dram_tensor("out_sub", [n_img, NR, NE], BF16, kind="ExternalOutput")
    out_o = nc.dram_tensor("out_obj", [n_img, NR, NE], BF16, kind="ExternalOutput")

    with tile.TileContext(nc) as tc:
        with (
            tc.tile_pool(name="singles", bufs=1) as singles,
            tc.tile_pool(name="io", bufs=4) as io,
            tc.tile_pool(name="pre", bufs=3) as pre,
            tc.tile_pool(name="col", bufs=3) as col,
            tc.tile_pool(name="rep", bufs=2) as rep,
            tc.tile_pool(name="mm", bufs=2) as mm,
            tc.tile_pool(name="mp", bufs=3) as mp,
            tc.tile_pool(name="ps", bufs=2, space="PSUM") as ps,
            tc.tile_pool(name="psd", bufs=4, space="PSUM") as psd,
            tc.tile_pool(name="dr", bufs=2, space="DRAM") as dr,
        ):
            ident = singles.tile([128, 128], F32, tag="ident")
            make_identity(nc, ident)
            identb = singles.tile([128, 128], BF16, tag="identb")
            nc.vector.tensor_copy(out=identb, in_=ident)

            for b in range(n_img):
                _build_image(nc, b, locals())
    if split_waits:
        _split_waits(nc)
    return nc


def _build_image(nc, b, env):
    io, pre, col, rep, mm, mp, ps, psd, dr = (env[k] for k in
        ("io", "pre", "col", "rep", "mm", "mp", "ps", "psd", "dr"))
    ident, identb = env["ident"], env["identb"]
    pb, pl, rol, rsl, rob, rsb, rv, tsz = (env[k] for k in
        ("pb", "pl", "rol", "rsl", "rob", "rsb", "rv", "tsz"))
    out_s, out_o = env["out_s"], env["out_o"]

    # image scalars W, H broadcast to all partitions
    Wt = col.tile([128, 1], F32, tag="Wt")
    Ht = col.tile([128, 1], F32, tag="Ht")
    nc.sync.dma_start(out=Wt, in_=_bcast(tsz[b, 1:2], 128))
    nc.sync.dma_start(out=Ht, in_=_bcast(tsz[b, 0:1], 128))

    # --- entity boxes: normalized xyxy + area + PIXEL centers -------------
    # PG cols: 0 x0 | 1 y0 | 2 x1 | 3 y1 | 4 area | 5 cxp | 6 cyp | 7 score
    EBT = io.tile([P, NCH, 4], F32, tag="EBT")
    nc.sync.dma_start(out=EBT, in_=pb[b].rearrange("(j p) c -> p j c", p=P))
    PG = pre.tile([P, NCH, 8], F32, tag="PG")
    nc.vector.scalar_tensor_tensor(
        out=PG[:, :, 0:2], in0=EBT[:, :, 2:4], scalar=-0.5, in1=EBT[:, :, 0:2],
        op0=OP.mult, op1=OP.add)
    nc.vector.scalar_tensor_tensor(
        out=PG[:, :, 2:4], in0=EBT[:, :, 2:4], scalar=0.5, in1=EBT[:, :, 0:2],
        op0=OP.mult, op1=OP.add)
    nc.vector.tensor_tensor(out=PG[:, :, 4:5], in0=EBT[:, :, 2:3],
                            in1=EBT[:, :, 3:4], op=OP.mult)
    nc.vector.tensor_scalar(out=PG[:, :, 5:6], in0=EBT[:, :, 0:1],
                            scalar1=Wt[:P], scalar2=None, op0=OP.mult)
    nc.vector.tensor_scalar(out=PG[:, :, 6:7], in0=EBT[:, :, 1:2],
                            scalar1=Ht[:P], scalar2=None, op0=OP.mult)

    # --- rel boxes: normalized xyxy + area --------------------------------
    def rel_box_prep(dram, tag):
        BT = io.tile([P, NCH, 4], F32, tag="BT")
        nc.sync.dma_start(out=BT, in_=dram[b].rearrange("(j p) c -> p j c", p=P))
        RB = col.tile([P, NCH, 5], F32, tag=tag)
        nc.vector.scalar_tensor_tensor(
            out=RB[:, :, 0:2], in0=BT[:, :, 2:4], scalar=-0.5, in1=BT[:, :, 0:2],
            op0=OP.mult, op1=OP.add)
        nc.vector.scalar_tensor_tensor(
            out=RB[:, :, 2:4], in0=BT[:, :, 2:4], scalar=0.5, in1=BT[:, :, 0:2],
            op0=OP.mult, op1=OP.add)
        nc.vector.tensor_tensor(out=RB[:, :, 4:5], in0=BT[:, :, 2:3],
                                in1=BT[:, :, 3:4], op=OP.mult)
        return RB

    RB_S = rel_box_prep(rsb, "RB_S")
    RB_O = rel_box_prep(rob, "RB_O")

    # rel_vec: negated pixel coords
    RVt = io.tile([P, NCH, 4], F32, tag="RVt")
    nc.sync.dma_start(out=RVt, in_=rv[b].rearrange("(j p) c -> p j c", p=P))
    VN = col.tile([P, NCH, 4], F32, tag="VN")
    SC4 = col.tile([128, 4], F32, tag="SC4")
    nc.vector.tensor_copy(out=SC4[:, 0:1], in_=Wt)
    nc.vector.tensor_copy(out=SC4[:, 1:2], in_=Ht)
    nc.vector.tensor_copy(out=SC4[:, 2:3], in_=Wt)
    nc.vector.tensor_copy(out=SC4[:, 3:4], in_=Ht)
    for j in range(NCH):
        nc.vector.tensor_tensor(out=VN[:, j, :], in0=RVt[:, j, :],
                                in1=SC4[:P], op=OP.mult)
    nc.vector.tensor_scalar(out=VN, in0=VN, scalar1=-1.0, scalar2=None,
                            op0=OP.mult)

    # --- softmax + packs (bf16) + PE transposes ---------------------------
    RHS_A = mm.tile([128, NE], BF16, tag="RHS_A")
    RHS_B = mm.tile([24, NE], BF16, tag="RHS_B")
    LS_A = mm.tile([128, NR], BF16, tag="LS_A")
    LS_B = mm.tile([24, NR], BF16, tag="LS_B")
    LO_A = mm.tile([128, NR], BF16, tag="LO_A")
    LO_B = mm.tile([24, NR], BF16, tag="LO_B")

    for t, (ldram, dA, dB) in {
        "ent": (pl, RHS_A, RHS_B),
        "rs": (rsl, LS_A, LS_B),
        "ro": (rol, LO_A, LO_B),
    }.items():
        for j in range(NCH):
            L = io.tile([P, NC1], F32, tag="L")
            nc.sync.dma_start(out=L, in_=ldram[b, P * j:P * (j + 1), :])
            E = pre.tile([P, NC1], F32, tag="E")
            sumc = col.tile([P, 1], F32, tag="sumc")
            nc.scalar.activation(out=E, in_=L, func=AF.Exp, accum_out=sumc)
            r = col.tile([P, 1], F32, tag="r")
            nc.vector.reciprocal(r, sumc)
            SQ = pre.tile([P, NCL], F32, tag="SQ")
            sqc = col.tile([P, 1], F32, tag="sqc")
            nc.scalar.activation(out=SQ, in_=E[:, :NCL], func=AF.Square,
                                 accum_out=sqc)
            PK = pre.tile([P, 152], BF16, tag="PK")
            if t == "ent":
                nc.scalar.activation(out=PK[:, 0:NCL], in_=E[:, :NCL],
                                     func=AF.Copy, scale=r)
                r2 = col.tile([P, 1], F32, tag="r2")
                nc.vector.tensor_tensor(out=r2, in0=r, in1=r, op=OP.mult)
                with nc.allow_low_precision(reason="pack bf16"):
                    nc.vector.tensor_tensor(out=PK[:, 151:152], in0=sqc,
                                            in1=r2, op=OP.mult)
                nc.vector.memset(PK[:, 150:151], 0.25)
                mx = col.tile([P, 1], F32, tag="mx")
                nc.vector.tensor_reduce(out=mx, in_=E[:, :NCL],
                                        axis=mybir.AxisListType.X, op=OP.max)
                nc.vector.tensor_tensor(out=PG[:, j, 7:8], in0=mx, in1=r,
                                        op=OP.mult)
            else:
                rm2 = col.tile([P, 1], F32, tag="rm2")
                nc.vector.tensor_scalar(out=rm2, in0=r, scalar1=-2.0,
                                        scalar2=None, op0=OP.mult)
                nc.scalar.activation(out=PK[:, 0:NCL], in_=E[:, :NCL],
                                     func=AF.Copy, scale=rm2)
                m4 = col.tile([P, 1], F32, tag="m4")
                nc.vector.tensor_tensor(out=m4, in0=rm2, in1=rm2, op=OP.mult)
                with nc.allow_low_precision(reason="pack bf16"):
                    nc.vector.tensor_tensor(out=PK[:, 150:151], in0=sqc,
                                            in1=m4, op=OP.mult)
                nc.vector.memset(PK[:, 151:152], 1.0)
            TA = ps.tile([128, P], BF16, tag="TT")
            with nc.allow_low_precision(reason="bf16 transpose"):
                nc.tensor.transpose(TA, PK[:, 0:128], identb[:P, :P])
            nc.scalar.copy(out=dA[:, P * j:P * (j + 1)], in_=TA)
            TB = ps.tile([24, P], BF16, tag="TT")
            with nc.allow_low_precision(reason="bf16 transpose"):
                nc.tensor.transpose(TB, PK[:, 128:152], identb[:P, :P])
            nc.scalar.copy(out=dB[:, P * j:P * (j + 1)], in_=TB)

    # --- entity rows -> DRAM -> broadcast ---------------------------------
    EROWS = rep.tile([8, NE], F32, tag="EROWS")
    for j in range(NCH):
        TE = ps.tile([8, P], F32, tag="TT")
        nc.tensor.transpose(TE, PG[:, j, :], ident[:P, :P])
        nc.vector.tensor_copy(out=EROWS[:, P * j:P * (j + 1)], in_=TE)
    EDRAM = dr.tile([8, NE], F32, tag="EDRAM", name="EDRAM")
    nc.sync.dma_start(out=EDRAM, in_=EROWS)
    names = ["X0R", "Y0R", "X1R", "Y1R", "AREAR", "CXPR", "CYPR", "SCRF"]
    RALL = rep.tile([128, 8, NE], F32, tag="RALL", name="RALL")
    nc.sync.dma_start(
        out=RALL,
        in_=bass.AP(tensor=EDRAM.tensor, offset=EDRAM.offset,
                    ap=[[0, 128]] + list(EDRAM.ap)))
    R = {nm_: RALL[:, k, :] for k, nm_ in enumerate(names)}
    RH = {}
    for nm_ in ("X0R", "Y0R", "X1R", "Y1R"):
        RH[nm_] = rep.tile([128, NE], FP16, tag="H" + nm_, name="H" + nm_)
        nc.scalar.activation(out=RH[nm_], in_=R[nm_], func=AF.Copy)
    SCRN = rep.tile([128, NE], FP16, tag="SCRN")
    nc.vector.tensor_scalar(out=SCRN, in0=R["SCRF"], scalar1=-1.0,
                            scalar2=None, op0=OP.mult)

    # --- map pipeline per (map, chunk) ------------------------------------
    for (lA, lB, RB, vxc, vyc, odram) in (
        (LS_A, LS_B, RB_S, 0, 1, out_s),
        (LO_A, LO_B, RB_O, 2, 3, out_o),
    ):
        OIMG = mp.tile([P, NCH, NE], BF16, tag="OIMG", name="OIMG")
        for j in range(NCH):
            rx0 = RB[:, j, 0:1]
            ry0 = RB[:, j, 1:2]
            rx1 = RB[:, j, 2:3]
            ry1 = RB[:, j, 3:4]
            rarea = RB[:, j, 4:5]
            nvx = VN[:, j, vxc:vxc + 1]
            nvy = VN[:, j, vyc:vyc + 1]

            D2 = psd.tile([P, NE], F32, tag="D2")
            with nc.allow_low_precision(reason="bf16 matmul"):
                nc.tensor.matmul(D2, lhsT=lA[:, P * j:P * (j + 1)], rhs=RHS_A,
                                 start=True, stop=False)
                nc.tensor.matmul(D2, lhsT=lB[:, P * j:P * (j + 1)], rhs=RHS_B,
                                 start=False, stop=True)

            def mh(tag):
                return mp.tile([P, NE], FP16, tag=tag, name=tag)

            def mb(tag):
                return mp.tile([P, NE], BF16, tag=tag, name=tag)

            with nc.allow_low_precision(reason="fp16 map pipeline"):
                # ACT: relu/abs/affine legs (fp16 outs), sqrt of D2
                wx = mh("wx")
                nc.scalar.activation(out=wx, in_=R["X1R"][:P], func=AF.Relu,
                                     bias=rx1, scale=-1.0)
                wy = mh("wy")
                nc.scalar.activation(out=wy, in_=R["Y1R"][:P], func=AF.Relu,
                                     bias=ry1, scale=-1.0)
                u2x = mh("u2x")
                nc.scalar.activation(out=u2x, in_=R["X0R"][:P], func=AF.Relu,
                                     bias=rx0, scale=-1.0)
                u2y = mh("u2y")
                nc.scalar.activation(out=u2y, in_=R["Y0R"][:P], func=AF.Relu,
                                     bias=ry0, scale=-1.0)
                SA = mh("SA")
                nc.scalar.activation(out=SA, in_=R["AREAR"][:P], func=AF.Relu,
                                     bias=rarea)
                ax = mh("ax")
                nc.scalar.activation(out=ax, in_=R["CXPR"][:P], func=AF.Abs,
                                     bias=nvx)
                ay = mh("ay")
                nc.scalar.activation(out=ay, in_=R["CYPR"][:P], func=AF.Abs,
                                     bias=nvy)
                s = mh("s")
                nc.scalar.activation(out=s, in_=D2, func=AF.Sqrt)

                # DVE dual-scalar TS (f32 rows in, fp16 out)
                pxs = mh("pxs")
                nc.vector.tensor_scalar(out=pxs, in0=RH["X0R"][:P], scalar1=rx0,
                                        scalar2=rx1, op0=OP.max, op1=OP.subtract)
                pys = mh("pys")
                nc.vector.tensor_scalar(out=pys, in0=RH["Y0R"][:P], scalar1=ry0,
                                        scalar2=ry1, op0=OP.max, op1=OP.subtract)
                u1x = mh("u1x")
                nc.vector.tensor_scalar(out=u1x, in0=RH["X1R"][:P], scalar1=rx1,
                                        scalar2=rx0, op0=OP.max, op1=OP.subtract)
                u1y = mh("u1y")
                nc.vector.tensor_scalar(out=u1y, in0=RH["Y1R"][:P], scalar1=ry1,
                                        scalar2=ry0, op0=OP.max, op1=OP.subtract)

                # fp16 TT chain
                dxn = mh("dxn")
                nc.vector.tensor_tensor(out=dxn, in0=pxs, in1=wx, op=OP.add)
                dyn = mh("dyn")
                nc.vector.tensor_tensor(out=dyn, in0=pys, in1=wy, op=OP.add)
                i1 = mh("i1")
                nc.vector.scalar_tensor_tensor(out=i1, in0=dxn, scalar=0.0,
                                               in1=dyn, op0=OP.min, op1=OP.mult)
                e2n = mh("e2n")
                nc.vector.tensor_scalar(out=e2n, in0=i1, scalar1=0.0,
                                        scalar2=-2.0, op0=OP.max, op1=OP.mult)
                un = mh("un")
                nc.vector.scalar_tensor_tensor(out=un, in0=i1, scalar=0.0,
                                               in1=SA, op0=OP.max, op1=OP.subtract)
                wc = mh("wc")
                nc.vector.tensor_tensor(out=wc, in0=u1x, in1=u2x, op=OP.add)
                hc = mh("hc")
                nc.vector.tensor_tensor(out=hc, in0=u1y, in1=u2y, op=OP.add)
                areac = mh("areac")
                nc.vector.tensor_tensor(out=areac, in0=wc, in1=hc, op=OP.mult)
                G2 = mh("G2")
                nc.vector.tensor_tensor(out=G2, in0=SA, in1=e2n, op=OP.add)
                U2 = mh("U2")
                nc.vector.tensor_tensor(out=U2, in0=un, in1=un, op=OP.mult)
                AG = mh("AG")
                nc.vector.tensor_tensor(out=AG, in0=areac, in1=G2, op=OP.mult)
                Nn = mh("Nn")
                nc.vector.tensor_tensor(out=Nn, in0=U2, in1=AG, op=OP.subtract)
                P1n = mh("P1n")
                nc.vector.tensor_tensor(out=P1n, in0=un, in1=areac, op=OP.mult)
                ds1 = mh("ds1")
                nc.vector.scalar_tensor_tensor(out=ds1, in0=ax, scalar=1.0,
                                               in1=ay, op0=OP.add, op1=OP.add)
                den = mh("den")
                nc.vector.scalar_tensor_tensor(out=den, in0=s, scalar=1.0,
                                               in1=ds1, op0=OP.add, op1=OP.mult)
                D3n = mb("D3n")
                nc.vector.tensor_tensor(out=D3n, in0=P1n, in1=den, op=OP.mult)
                r3n = mb("r3n")
                nc.vector.reciprocal(out=r3n, in_=D3n)
                nm_t = mb("nm")
                nc.vector.scalar_tensor_tensor(out=nm_t, in0=Nn, scalar=0.0,
                                               in1=r3n, op0=OP.max, op1=OP.mult)
                nc.vector.tensor_tensor(out=OIMG[:, j, :], in0=nm_t,
                                        in1=SCRN[:P], op=OP.mult)
        nc.sync.dma_start(
            out=odram[b].rearrange("(j p) c -> p j c", p=P), in_=OIMG)


class _CompiledKernel:
    """Compiled SPMD executable: jit built once, reusable across calls."""

    def __init__(self, nc, n_cores):
        import jax
        from jax.sharding import Mesh, PartitionSpec
        try:
            from jax.experimental.shard_map import shard_map
        except Exception:
            from jax.shard_map import shard_map
        from concourse import bass2jax
        from concourse.bass2jax import _bass_exec_p, install_neuronx_cc_hook

        install_neuronx_cc_hook()
        self.jax = jax
        self.n_cores = n_cores
        partition_name = (nc.partition_id_tensor.name
                          if nc.partition_id_tensor else None)
        in_names, out_names, out_avals, zero_outs = [], [], [], []
        for alloc in nc.m.functions[0].allocations:
            if not isinstance(alloc, mybir.MemoryLocationSet):
                continue
            name = alloc.memorylocations[0].name
            if alloc.kind == "ExternalInput":
                if name != partition_name:
                    in_names.append(name)
            elif alloc.kind == "ExternalOutput":
                shape = tuple(alloc.tensor_shape)
                dtype = mybir.dt.np(alloc.dtype)
                out_names.append(name)
                out_avals.append(jax.core.ShapedArray(shape, dtype))
                zero_outs.append(np.zeros(shape, dtype))
        self.in_names = in_names
        self.out_names = out_names
        self.out_avals = out_avals
        self.zero_outs = zero_outs
        all_in = in_names + out_names
        if partition_name is not None:
            all_in.append(partition_name)

        def _body(*args):
            operands = list(args)
            if partition_name is not None:
                operands.append(bass2jax.partition_id_tensor())
            return tuple(_bass_exec_p.bind(
                *operands,
                out_avals=tuple(out_avals),
                in_names=tuple(all_in),
                out_names=tuple(out_names),
                lowering_input_output_aliases=(),
                sim_require_finite=True,
                sim_require_nnan=True,
                nc=nc,
            ))

        devices = jax.devices()[:n_cores]
        self._mesh = Mesh(np.asarray(devices), ("core",))
        nin = len(in_names) + len(out_names)
        self._fn = jax.jit(
            shard_map(_body, mesh=self._mesh,
                      in_specs=(PartitionSpec("core"),) * nin,
                      out_specs=(PartitionSpec("core"),) * len(out_names),
                      check_rep=False),
            keep_unused=True)

    def run(self, in_maps):
        jax = self.jax
        n = self.n_cores
        per_core = [[np.asarray(m[nm]) for nm in self.in_names]
                    for m in in_maps]
        concat_in = [np.concatenate([per_core[c][i] for c in range(n)], axis=0)
                     for i in range(len(self.in_names))]
        concat_zero = [np.zeros((n * z.shape[0], *z.shape[1:]), z.dtype)
                       for z in self.zero_outs]
        outs = jax.block_until_ready(self._fn(*concat_in, *concat_zero))
        return [
            {nm: np.asarray(outs[i]).reshape(n, *self.out_avals[i].shape)[c]
             for i, nm in enumerate(self.out_names)}
            for c in range(n)
        ]


_CACHE = {}


def _get_nc():
    if "nc" not in _CACHE:
        _CACHE["nc"] = _build(N_IMG)
    return _CACHE["nc"]


def _get_ck():
    if "ck" not in _CACHE:
        _CACHE["ck"] = _CompiledKernel(_get_nc(), N_CORES)
    return _CACHE["ck"]


def kernel(pred_boxes, pred_logits, pred_rel_obj_logits, pred_rel_sub_logits,
           pred_rel_obj_box, pred_rel_sub_box, pred_rel_vec, target_sizes):
    inp = {
        "pred_boxes": np.ascontiguousarray(pred_boxes, dtype=np.float32),
        "pred_logits": np.ascontiguousarray(pred_logits, dtype=np.float32),
        "pred_rel_obj_logits": np.ascontiguousarray(pred_rel_obj_logits, dtype=np.float32),
        "pred_rel_sub_logits": np.ascontiguousarray(pred_rel_sub_logits, dtype=np.float32),
        "pred_rel_obj_box": np.ascontiguousarray(pred_rel_obj_box, dtype=np.float32),
        "pred_rel_sub_box": np.ascontiguousarray(pred_rel_sub_box, dtype=np.float32),
        "pred_rel_vec": np.ascontiguousarray(pred_rel_vec, dtype=np.float32),
        "target_sizes": np.ascontiguousarray(target_sizes, dtype=np.float32),
    }
    in_maps = [{k: v[c * N_IMG:(c + 1) * N_IMG] for k, v in inp.items()}
               for c in range(N_CORES)]
    res = None
    try:
        res = _get_ck().run(in_maps)
    except Exception:
        import time as _time
        _time.sleep(2.0)
        try:
            res = _get_ck().run(in_maps)
        except Exception:
            r = bass_utils.run_bass_kernel_spmd(
                _get_nc(), in_maps, core_ids=list(range(N_CORES)))
            res = r.results
    sub = np.concatenate([res[c]["out_sub"] for c in range(N_CORES)],
                         axis=0).astype(np.float32)
    obj = np.concatenate([res[c]["out_obj"] for c in range(N_CORES)],
                         axis=0).astype(np.float32)
    return sub, obj


# revision 5
# speedup vs baseline: 1.3447x; 1.3447x over previous
"""Trainium2 Bass kernel for nn_EntitiesIndexingHeadRuleBased (nms_detection).

v3: fp16 TT-form map pipeline (Pool has no ALU ops on real HW).

v2: engine-balanced, bf16-heavy rewrite.
  * GIoU is scale-invariant -> box chain runs on NORMALIZED coords (f32
    through the interval cancellations, bf16 after).
  * L1 chain needs pixel scale: centers kept f32 until after |.|, bf16 after.
  * cls chain: softmax packs in bf16, PE matmuls bf16 (3x faster), sqrt on ACT.
  * Work split per (map,chunk): ACT 7 ops (relu/abs/affine/sqrt),
    POOL(gpsimd) 8 stt combines, DVE 5 dual-scalar TS + 6 stt + 3 TT + recip,
    PE 2 bf16 matmuls.
  * Outputs written bf16, host converts to f32.

Sharding: batch 64 = 8 images x 8 cores, SPMD.
"""
import sys
sys.path.insert(0, '/opt/trn_rl_repo')

import numpy as np
import bass_rust
import concourse.bass as bass
import concourse.tile as tile
import concourse.tile as tile_mod
from concourse import mybir
from concourse import bass_utils
from concourse.masks import make_identity
from concourse.tile import TileContext

F32 = mybir.dt.float32
BF16 = mybir.dt.bfloat16
FP16 = mybir.dt.float16
AF = mybir.ActivationFunctionType
OP = mybir.AluOpType

B = 64
NE = 500
NR = 500
NC1 = 151
NCL = 150
P = 125
NCH = 4
N_CORES = 8
N_IMG = B // N_CORES
# input blob layout (f32 elements per image)
OFF_PB, OFF_PL, OFF_RSL, OFF_ROL = 0, 2000, 77500, 153000
OFF_RSB, OFF_ROB, OFF_RV, OFF_TSZ = 228500, 230500, 232500, 234500
BLOB = 234502

# ---------------------------------------------------------------------------
# Walrus workaround: instructions may carry at most one sync-wait command.
# ---------------------------------------------------------------------------

_MAXW = 1


def _patched_drain_and_barrier(self, tick_clock, wait_clock):
    ScopedClock = tile_mod.ScopedClock
    carrier = self.nc.sync.nop(nofuse=True)
    wait_clock.add_sem_waits(carrier.ins,
                             ScopedClock({None: tick_clock.global_clock}))
    si = carrier.ins.sync_info
    waits = list(si.on_wait) if si is not None else []
    if len(waits) > _MAXW:
        carrier.ins.sync_info = bass_rust.SyncInfo(
            on_wait=waits[:_MAXW], on_update=[])
        for i in range(_MAXW, len(waits), _MAXW):
            nop = self.nc.sync.nop(nofuse=True)
            nop.ins.sync_info = bass_rust.SyncInfo(
                on_wait=waits[i:i + _MAXW], on_update=[])
    self.nc.sync.drain()
    self.nc.all_engine_barrier()
    assert self.sems is not None
    popped = self.nc._tile_sem_poison_stack.pop()
    assert popped is self._sem_poison
    self.nc.clear_and_free_semaphores(list(self.sems.allocated().values()))
    self.nc.all_engine_barrier()


TileContext._drain_and_barrier = _patched_drain_and_barrier


def _split_waits(nc, maxw=_MAXW):
    for fn in nc.m.functions:
        for blk in fn.blocks:
            newl = []
            changed = False
            for ins in blk.instructions:
                si = ins.sync_info
                waits = list(si.on_wait) if si is not None else []
                if len(waits) > maxw:
                    changed = True
                    carried, rest = waits[:-maxw], waits[-maxw:]
                    for i in range(0, len(carried), maxw):
                        nop = mybir.InstNoOp(
                            name=f"{ins.name}-sw{i}",
                            sync_info=mybir.SyncInfo(
                                on_wait=carried[i:i + maxw], on_update=[]),
                            bass_nofuse=True,
                            engine=ins.engine,
                        )
                        newl.append(nop)
                    ins.sync_info = mybir.SyncInfo(
                        on_wait=rest, on_update=list(si.on_update))
                newl.append(ins)
            if changed:
                blk.instructions = newl


# ---------------------------------------------------------------------------
# Kernel builder
# ---------------------------------------------------------------------------

def _bcast(ap, p):
    return bass.AP(tensor=ap.tensor, offset=ap.offset,
                   ap=[[0, p]] + list(ap.ap[1:]))


def _build(n_img, split_waits=True):
    nc = bass.Bass("TRN2", target_bir_lowering=False)

    inblob = nc.dram_tensor("inblob", [n_img, BLOB], F32, kind="ExternalInput")
    outblob = nc.dram_tensor("outblob", [n_img, 2, NR, NE], BF16,
                             kind="ExternalOutput")

    with tile.TileContext(nc) as tc:
        with (
            tc.tile_pool(name="singles", bufs=1) as singles,
            tc.tile_pool(name="io", bufs=4) as io,
            tc.tile_pool(name="pre", bufs=3) as pre,
            tc.tile_pool(name="col", bufs=3) as col,
            tc.tile_pool(name="rep", bufs=2) as rep,
            tc.tile_pool(name="mm", bufs=2) as mm,
            tc.tile_pool(name="mp", bufs=3) as mp,
            tc.tile_pool(name="ps", bufs=2, space="PSUM") as ps,
            tc.tile_pool(name="psd", bufs=4, space="PSUM") as psd,
            tc.tile_pool(name="dr", bufs=2, space="DRAM") as dr,
        ):
            ident = singles.tile([128, 128], F32, tag="ident")
            make_identity(nc, ident)
            identb = singles.tile([128, 128], BF16, tag="identb")
            nc.vector.tensor_copy(out=identb, in_=ident)

            for b in range(n_img):
                _build_image(nc, b, locals())
    if split_waits:
        _split_waits(nc)
    return nc


def _build_image(nc, b, env):
    io, pre, col, rep, mm, mp, ps, psd, dr = (env[k] for k in
        ("io", "pre", "col", "rep", "mm", "mp", "ps", "psd", "dr"))
    ident, identb = env["ident"], env["identb"]
    inblob, outblob = env["inblob"], env["outblob"]

    # image scalars W, H broadcast to all partitions
    Wt = col.tile([128, 1], F32, tag="Wt")
    Ht = col.tile([128, 1], F32, tag="Ht")
    nc.sync.dma_start(out=Wt, in_=_bcast(inblob[b, OFF_TSZ + 1:OFF_TSZ + 2], 128))
    nc.sync.dma_start(out=Ht, in_=_bcast(inblob[b, OFF_TSZ:OFF_TSZ + 1], 128))

    # --- entity boxes: normalized xyxy + area + PIXEL centers -------------
    # PG cols: 0 x0 | 1 y0 | 2 x1 | 3 y1 | 4 area | 5 cxp | 6 cyp | 7 score
    EBT = io.tile([P, NCH, 4], F32, tag="EBT")
    nc.sync.dma_start(out=EBT, in_=inblob[b, OFF_PB:OFF_PB + 2000].rearrange("(j p c) -> p j c", p=P, c=4))
    PG = pre.tile([P, NCH, 8], F32, tag="PG")
    nc.vector.scalar_tensor_tensor(
        out=PG[:, :, 0:2], in0=EBT[:, :, 2:4], scalar=-0.5, in1=EBT[:, :, 0:2],
        op0=OP.mult, op1=OP.add)
    nc.vector.scalar_tensor_tensor(
        out=PG[:, :, 2:4], in0=EBT[:, :, 2:4], scalar=0.5, in1=EBT[:, :, 0:2],
        op0=OP.mult, op1=OP.add)
    nc.vector.tensor_tensor(out=PG[:, :, 4:5], in0=EBT[:, :, 2:3],
                            in1=EBT[:, :, 3:4], op=OP.mult)
    nc.vector.tensor_scalar(out=PG[:, :, 5:6], in0=EBT[:, :, 0:1],
                            scalar1=Wt[:P], scalar2=None, op0=OP.mult)
    nc.vector.tensor_scalar(out=PG[:, :, 6:7], in0=EBT[:, :, 1:2],
                            scalar1=Ht[:P], scalar2=None, op0=OP.mult)

    # --- rel boxes: normalized xyxy + area --------------------------------
    def rel_box_prep(off, tag):
        BT = io.tile([P, NCH, 4], F32, tag="BT")
        nc.sync.dma_start(out=BT, in_=inblob[b, off:off + 2000]
                          .rearrange("(j p c) -> p j c", p=P, c=4))
        RB = col.tile([P, NCH, 5], F32, tag=tag)
        nc.vector.scalar_tensor_tensor(
            out=RB[:, :, 0:2], in0=BT[:, :, 2:4], scalar=-0.5, in1=BT[:, :, 0:2],
            op0=OP.mult, op1=OP.add)
        nc.vector.scalar_tensor_tensor(
            out=RB[:, :, 2:4], in0=BT[:, :, 2:4], scalar=0.5, in1=BT[:, :, 0:2],
            op0=OP.mult, op1=OP.add)
        nc.vector.tensor_tensor(out=RB[:, :, 4:5], in0=BT[:, :, 2:3],
                                in1=BT[:, :, 3:4], op=OP.mult)
        return RB

    RB_S = rel_box_prep(OFF_RSB, "RB_S")
    RB_O = rel_box_prep(OFF_ROB, "RB_O")

    # rel_vec: negated pixel coords
    RVt = io.tile([P, NCH, 4], F32, tag="RVt")
    nc.sync.dma_start(out=RVt, in_=inblob[b, OFF_RV:OFF_RV + 2000].rearrange("(j p c) -> p j c", p=P, c=4))
    VN = col.tile([P, NCH, 4], F32, tag="VN")
    SC4 = col.tile([128, 4], F32, tag="SC4")
    nc.vector.tensor_copy(out=SC4[:, 0:1], in_=Wt)
    nc.vector.tensor_copy(out=SC4[:, 1:2], in_=Ht)
    nc.vector.tensor_copy(out=SC4[:, 2:3], in_=Wt)
    nc.vector.tensor_copy(out=SC4[:, 3:4], in_=Ht)
    for j in range(NCH):
        nc.vector.tensor_tensor(out=VN[:, j, :], in0=RVt[:, j, :],
                                in1=SC4[:P], op=OP.mult)
    nc.vector.tensor_scalar(out=VN, in0=VN, scalar1=-1.0, scalar2=None,
                            op0=OP.mult)

    # --- softmax + packs (bf16) + PE transposes ---------------------------
    RHS_A = mm.tile([128, NE], BF16, tag="RHS_A")
    RHS_B = mm.tile([24, NE], BF16, tag="RHS_B")
    LS_A = mm.tile([128, NR], BF16, tag="LS_A")
    LS_B = mm.tile([24, NR], BF16, tag="LS_B")
    LO_A = mm.tile([128, NR], BF16, tag="LO_A")
    LO_B = mm.tile([24, NR], BF16, tag="LO_B")

    for t, (loff, dA, dB) in {
        "ent": (OFF_PL, RHS_A, RHS_B),
        "rs": (OFF_RSL, LS_A, LS_B),
        "ro": (OFF_ROL, LO_A, LO_B),
    }.items():
        for j in range(NCH):
            L = io.tile([P, NC1], F32, tag="L")
            lo = loff + P * j * NC1
            nc.sync.dma_start(out=L, in_=inblob[b, lo:lo + P * NC1]
                              .rearrange("(p c) -> p c", p=P))
            E = pre.tile([P, NC1], F32, tag="E")
            sumc = col.tile([P, 1], F32, tag="sumc")
            nc.scalar.activation(out=E, in_=L, func=AF.Exp, accum_out=sumc)
            r = col.tile([P, 1], F32, tag="r")
            nc.vector.reciprocal(r, sumc)
            SQ = pre.tile([P, NCL], F32, tag="SQ")
            sqc = col.tile([P, 1], F32, tag="sqc")
            nc.scalar.activation(out=SQ, in_=E[:, :NCL], func=AF.Square,
                                 accum_out=sqc)
            PK = pre.tile([P, 152], BF16, tag="PK")
            if t == "ent":
                nc.scalar.activation(out=PK[:, 0:NCL], in_=E[:, :NCL],
                                     func=AF.Copy, scale=r)
                r2 = col.tile([P, 1], F32, tag="r2")
                nc.vector.tensor_tensor(out=r2, in0=r, in1=r, op=OP.mult)
                with nc.allow_low_precision(reason="pack bf16"):
                    nc.vector.tensor_tensor(out=PK[:, 151:152], in0=sqc,
                                            in1=r2, op=OP.mult)
                nc.vector.memset(PK[:, 150:151], 0.25)
                mx = col.tile([P, 1], F32, tag="mx")
                nc.vector.tensor_reduce(out=mx, in_=E[:, :NCL],
                                        axis=mybir.AxisListType.X, op=OP.max)
                nc.vector.tensor_tensor(out=PG[:, j, 7:8], in0=mx, in1=r,
                                        op=OP.mult)
            else:
                rm2 = col.tile([P, 1], F32, tag="rm2")
                nc.vector.tensor_scalar(out=rm2, in0=r, scalar1=-2.0,
                                        scalar2=None, op0=OP.mult)
                nc.scalar.activation(out=PK[:, 0:NCL], in_=E[:, :NCL],
                                     func=AF.Copy, scale=rm2)
                m4 = col.tile([P, 1], F32, tag="m4")
                nc.vector.tensor_tensor(out=m4, in0=rm2, in1=rm2, op=OP.mult)
                with nc.allow_low_precision(reason="pack bf16"):
                    nc.vector.tensor_tensor(out=PK[:, 150:151], in0=sqc,
                                            in1=m4, op=OP.mult)
                nc.vector.memset(PK[:, 151:152], 1.0)
            TA = ps.tile([128, P], BF16, tag="TT")
            with nc.allow_low_precision(reason="bf16 transpose"):
                nc.tensor.transpose(TA, PK[:, 0:128], identb[:P, :P])
            nc.scalar.copy(out=dA[:, P * j:P * (j + 1)], in_=TA)
            TB = ps.tile([24, P], BF16, tag="TT")
            with nc.allow_low_precision(reason="bf16 transpose"):
                nc.tensor.transpose(TB, PK[:, 128:152], identb[:P, :P])
            nc.scalar.copy(out=dB[:, P * j:P * (j + 1)], in_=TB)

    # --- entity rows -> DRAM -> broadcast ---------------------------------
    EROWS = rep.tile([8, NE], F32, tag="EROWS")
    for j in range(NCH):
        TE = ps.tile([8, P], F32, tag="TT")
        nc.tensor.transpose(TE, PG[:, j, :], ident[:P, :P])
        nc.vector.tensor_copy(out=EROWS[:, P * j:P * (j + 1)], in_=TE)
    EDRAM = dr.tile([8, NE], F32, tag="EDRAM", name="EDRAM")
    nc.sync.dma_start(out=EDRAM, in_=EROWS)
    names = ["X0R", "Y0R", "X1R", "Y1R", "AREAR", "CXPR", "CYPR", "SCRF"]
    RALL = rep.tile([128, 8, NE], F32, tag="RALL", name="RALL")
    nc.sync.dma_start(
        out=RALL,
        in_=bass.AP(tensor=EDRAM.tensor, offset=EDRAM.offset,
                    ap=[[0, 128]] + list(EDRAM.ap)))
    R = {nm_: RALL[:, k, :] for k, nm_ in enumerate(names)}
    RH = {}
    for nm_ in ("X0R", "Y0R", "X1R", "Y1R"):
        RH[nm_] = rep.tile([128, NE], FP16, tag="H" + nm_, name="H" + nm_)
        nc.scalar.activation(out=RH[nm_], in_=R[nm_], func=AF.Copy)
    SCRN = rep.tile([128, NE], FP16, tag="SCRN")
    nc.vector.tensor_scalar(out=SCRN, in0=R["SCRF"], scalar1=-1.0,
                            scalar2=None, op0=OP.mult)

    # --- map pipeline per (map, chunk) ------------------------------------
    for (lA, lB, RB, vxc, vyc, om) in (
        (LS_A, LS_B, RB_S, 0, 1, 0),
        (LO_A, LO_B, RB_O, 2, 3, 1),
    ):
        OIMG = mp.tile([P, NCH, NE], BF16, tag="OIMG", name="OIMG")
        for j in range(NCH):
            rx0 = RB[:, j, 0:1]
            ry0 = RB[:, j, 1:2]
            rx1 = RB[:, j, 2:3]
            ry1 = RB[:, j, 3:4]
            rarea = RB[:, j, 4:5]
            nvx = VN[:, j, vxc:vxc + 1]
            nvy = VN[:, j, vyc:vyc + 1]

            D2 = psd.tile([P, NE], F32, tag="D2")
            with nc.allow_low_precision(reason="bf16 matmul"):
                nc.tensor.matmul(D2, lhsT=lA[:, P * j:P * (j + 1)], rhs=RHS_A,
                                 start=True, stop=False)
                nc.tensor.matmul(D2, lhsT=lB[:, P * j:P * (j + 1)], rhs=RHS_B,
                                 start=False, stop=True)

            def mh(tag):
                return mp.tile([P, NE], FP16, tag=tag, name=tag)

            def mb(tag):
                return mp.tile([P, NE], BF16, tag=tag, name=tag)

            with nc.allow_low_precision(reason="fp16 map pipeline"):
                # ACT: relu/abs/affine legs (fp16 outs), sqrt of D2
                wx = mh("wx")
                nc.scalar.activation(out=wx, in_=R["X1R"][:P], func=AF.Relu,
                                     bias=rx1, scale=-1.0)
                wy = mh("wy")
                nc.scalar.activation(out=wy, in_=R["Y1R"][:P], func=AF.Relu,
                                     bias=ry1, scale=-1.0)
                u2x = mh("u2x")
                nc.scalar.activation(out=u2x, in_=R["X0R"][:P], func=AF.Relu,
                                     bias=rx0, scale=-1.0)
                u2y = mh("u2y")
                nc.scalar.activation(out=u2y, in_=R["Y0R"][:P], func=AF.Relu,
                                     bias=ry0, scale=-1.0)
                SA = mh("SA")
                nc.scalar.activation(out=SA, in_=R["AREAR"][:P], func=AF.Relu,
                                     bias=rarea)
                ax = mh("ax")
                nc.scalar.activation(out=ax, in_=R["CXPR"][:P], func=AF.Abs,
                                     bias=nvx)
                ay = mh("ay")
                nc.scalar.activation(out=ay, in_=R["CYPR"][:P], func=AF.Abs,
                                     bias=nvy)
                s = mh("s")
                nc.scalar.activation(out=s, in_=D2, func=AF.Sqrt)

                # DVE dual-scalar TS (f32 rows in, fp16 out)
                pxs = mh("pxs")
                nc.vector.tensor_scalar(out=pxs, in0=RH["X0R"][:P], scalar1=rx0,
                                        scalar2=rx1, op0=OP.max, op1=OP.subtract)
                pys = mh("pys")
                nc.vector.tensor_scalar(out=pys, in0=RH["Y0R"][:P], scalar1=ry0,
                                        scalar2=ry1, op0=OP.max, op1=OP.subtract)
                u1x = mh("u1x")
                nc.vector.tensor_scalar(out=u1x, in0=RH["X1R"][:P], scalar1=rx1,
                                        scalar2=rx0, op0=OP.max, op1=OP.subtract)
                u1y = mh("u1y")
                nc.vector.tensor_scalar(out=u1y, in0=RH["Y1R"][:P], scalar1=ry1,
                                        scalar2=ry0, op0=OP.max, op1=OP.subtract)

                # fp16 TT chain
                dxn = mh("dxn")
                nc.vector.tensor_tensor(out=dxn, in0=pxs, in1=wx, op=OP.add)
                dyn = mh("dyn")
                nc.vector.tensor_tensor(out=dyn, in0=pys, in1=wy, op=OP.add)
                i1 = mh("i1")
                nc.vector.scalar_tensor_tensor(out=i1, in0=dxn, scalar=0.0,
                                               in1=dyn, op0=OP.min, op1=OP.mult)
                e2n = mh("e2n")
                nc.vector.tensor_scalar(out=e2n, in0=i1, scalar1=0.0,
                                        scalar2=-2.0, op0=OP.max, op1=OP.mult)
                un = mh("un")
                nc.vector.scalar_tensor_tensor(out=un, in0=i1, scalar=0.0,
                                               in1=SA, op0=OP.max, op1=OP.subtract)
                wc = mh("wc")
                nc.vector.tensor_tensor(out=wc, in0=u1x, in1=u2x, op=OP.add)
                hc = mh("hc")
                nc.vector.tensor_tensor(out=hc, in0=u1y, in1=u2y, op=OP.add)
                areac = mh("areac")
                nc.vector.tensor_tensor(out=areac, in0=wc, in1=hc, op=OP.mult)
                G2 = mh("G2")
                nc.vector.tensor_tensor(out=G2, in0=SA, in1=e2n, op=OP.add)
                U2 = mh("U2")
                nc.vector.tensor_tensor(out=U2, in0=un, in1=un, op=OP.mult)
                AG = mh("AG")
                nc.vector.tensor_tensor(out=AG, in0=areac, in1=G2, op=OP.mult)
                Nn = mh("Nn")
                nc.vector.tensor_tensor(out=Nn, in0=U2, in1=AG, op=OP.subtract)
                P1n = mh("P1n")
                nc.vector.tensor_tensor(out=P1n, in0=un, in1=areac, op=OP.mult)
                ds1 = mh("ds1")
                nc.vector.scalar_tensor_tensor(out=ds1, in0=ax, scalar=1.0,
                                               in1=ay, op0=OP.add, op1=OP.add)
                den = mh("den")
                nc.vector.scalar_tensor_tensor(out=den, in0=s, scalar=1.0,
                                               in1=ds1, op0=OP.add, op1=OP.mult)
                D3n = mb("D3n")
                nc.vector.tensor_tensor(out=D3n, in0=P1n, in1=den, op=OP.mult)
                r3n = mb("r3n")
                nc.vector.reciprocal(out=r3n, in_=D3n)
                nm_t = mb("nm")
                nc.vector.scalar_tensor_tensor(out=nm_t, in0=Nn, scalar=0.0,
                                               in1=r3n, op0=OP.max, op1=OP.mult)
                nc.vector.tensor_tensor(out=OIMG[:, j, :], in0=nm_t,
                                        in1=SCRN[:P], op=OP.mult)
        nc.sync.dma_start(
            out=outblob[b, om].rearrange("(j p) c -> p j c", p=P), in_=OIMG)


class _CompiledKernel:
    """Compiled SPMD executable: jit built once, reusable across calls."""

    def __init__(self, nc, n_cores):
        import jax
        from jax.sharding import Mesh, PartitionSpec
        try:
            from jax.experimental.shard_map import shard_map
        except Exception:
            from jax.shard_map import shard_map
        from concourse import bass2jax
        from concourse.bass2jax import _bass_exec_p, install_neuronx_cc_hook

        install_neuronx_cc_hook()
        self.jax = jax
        self.n_cores = n_cores
        partition_name = (nc.partition_id_tensor.name
                          if nc.partition_id_tensor else None)
        in_names, out_names, out_avals, zero_outs = [], [], [], []
        for alloc in nc.m.functions[0].allocations:
            if not isinstance(alloc, mybir.MemoryLocationSet):
                continue
            name = alloc.memorylocations[0].name
            if alloc.kind == "ExternalInput":
                if name != partition_name:
                    in_names.append(name)
            elif alloc.kind == "ExternalOutput":
                shape = tuple(alloc.tensor_shape)
                dtype = mybir.dt.np(alloc.dtype)
                out_names.append(name)
                out_avals.append(jax.core.ShapedArray(shape, dtype))
                zero_outs.append(np.zeros(shape, dtype))
        self.in_names = in_names
        self.out_names = out_names
        self.out_avals = out_avals
        self.zero_outs = zero_outs
        all_in = in_names + out_names
        if partition_name is not None:
            all_in.append(partition_name)

        def _body(*args):
            operands = list(args)
            if partition_name is not None:
                operands.append(bass2jax.partition_id_tensor())
            return tuple(_bass_exec_p.bind(
                *operands,
                out_avals=tuple(out_avals),
                in_names=tuple(all_in),
                out_names=tuple(out_names),
                lowering_input_output_aliases=(),
                sim_require_finite=True,
                sim_require_nnan=True,
                nc=nc,
            ))

        devices = jax.devices()[:n_cores]
        self._mesh = Mesh(np.asarray(devices), ("core",))
        nin = len(in_names) + len(out_names)
        self._fn = jax.jit(
            shard_map(_body, mesh=self._mesh,
                      in_specs=(PartitionSpec("core"),) * nin,
                      out_specs=(PartitionSpec("core"),) * len(out_names),
                      check_rep=False),
            keep_unused=True)

    def run(self, in_maps):
        jax = self.jax
        n = self.n_cores
        per_core = [[np.asarray(m[nm]) for nm in self.in_names]
                    for m in in_maps]
        concat_in = [np.concatenate([per_core[c][i] for c in range(n)], axis=0)
                     for i in range(len(self.in_names))]
        concat_zero = [np.zeros((n * z.shape[0], *z.shape[1:]), z.dtype)
                       for z in self.zero_outs]
        outs = jax.block_until_ready(self._fn(*concat_in, *concat_zero))
        return [
            {nm: np.asarray(outs[i]).reshape(n, *self.out_avals[i].shape)[c]
             for i, nm in enumerate(self.out_names)}
            for c in range(n)
        ]


_CACHE = {}


def _get_nc():
    if "nc" not in _CACHE:
        _CACHE["nc"] = _build(N_IMG)
    return _CACHE["nc"]


def _get_ck():
    if "ck" not in _CACHE:
        _CACHE["ck"] = _CompiledKernel(_get_nc(), N_CORES)
    return _CACHE["ck"]


def kernel(pred_boxes, pred_logits, pred_rel_obj_logits, pred_rel_sub_logits,
           pred_rel_obj_box, pred_rel_sub_box, pred_rel_vec, target_sizes):
    blob = make_blob(pred_boxes, pred_logits, pred_rel_obj_logits,
                     pred_rel_sub_logits, pred_rel_obj_box, pred_rel_sub_box,
                     pred_rel_vec, target_sizes)
    in_maps = [{"inblob": blob[c * N_IMG:(c + 1) * N_IMG]}
               for c in range(N_CORES)]
    res = None
    try:
        res = _get_ck().run(in_maps)
    except Exception:
        import time as _time
        _time.sleep(2.0)
        try:
            res = _get_ck().run(in_maps)
        except Exception:
            r = bass_utils.run_bass_kernel_spmd(
                _get_nc(), in_maps, core_ids=list(range(N_CORES)))
            res = r.results
    ob = np.concatenate([res[c]["outblob"] for c in range(N_CORES)], axis=0)
    return ob[:, 0].astype(np.float32), ob[:, 1].astype(np.float32)


def make_blob(pred_boxes, pred_logits, pred_rel_obj_logits,
              pred_rel_sub_logits, pred_rel_obj_box, pred_rel_sub_box,
              pred_rel_vec, target_sizes):
    blob = np.empty((B, BLOB), np.float32)
    blob[:, OFF_PB:OFF_PB + 2000] = np.asarray(pred_boxes, np.float32).reshape(B, -1)
    blob[:, OFF_PL:OFF_PL + 75500] = np.asarray(pred_logits, np.float32).reshape(B, -1)
    blob[:, OFF_RSL:OFF_RSL + 75500] = np.asarray(pred_rel_sub_logits, np.float32).reshape(B, -1)
    blob[:, OFF_ROL:OFF_ROL + 75500] = np.asarray(pred_rel_obj_logits, np.float32).reshape(B, -1)
    blob[:, OFF_RSB:OFF_RSB + 2000] = np.asarray(pred_rel_sub_box, np.float32).reshape(B, -1)
    blob[:, OFF_ROB:OFF_ROB + 2000] = np.asarray(pred_rel_obj_box, np.float32).reshape(B, -1)
    blob[:, OFF_RV:OFF_RV + 2000] = np.asarray(pred_rel_vec, np.float32).reshape(B, -1)
    blob[:, OFF_TSZ:OFF_TSZ + 2] = np.asarray(target_sizes, np.float32)
    return blob
